# revision 9
# baseline (speedup 1.0000x reference)
"""TRN2 Bass kernel for nn_MFILoss_38225208934871.

loss = sum((diag(S)-1)^2) + 0.2 * sum_i [ sum_j S_off[i,j]^3 / (mean_j S_off[i,j] + 1e-6) ]
where S = t_norm @ t_norm.T, t_norm = L2-row-normalized t_prime [8192, 768].

Strategy (8 NeuronCores, SPMD, symmetric-triangle formulation):
  S is symmetric, so S^3 (elementwise) is too: row_cube[i] needs every
  unordered pair {i, j} once.  Each 128x128 block-tile (r, q) with q > r is
  computed ONCE; its row-sums go to rows of block r, and its column-sums
  (accumulated in SBUF, partition-reduced by a ones-vector matmul at the
  end) go to rows of block q.  Diagonal blocks contribute row-sums only.
  This nearly halves TensorE work vs. the full [V/8, V] slab per core.

  Uniform SPMD program: 8 stationary "slots" per core; slot s computes the
  fixed column suffix [1024*s, 8192).  The 64 row-blocks are dealt to
  (core, slot) bins serpentine-style so slot s always holds a row-block r
  in [8s, 8s+8): the <=7 leading "pad" block-columns of each slot are
  duplicated lower-triangle work whose results the host discards (the
  first 1024 columns of every slot get per-block-column row-sums and a
  separate column-accumulator strip so real/pad/diagonal parts separate).

  mean_neg (the 1e-6-offset denominator) is computed on host in fp64
  exactly; the 128 rows with smallest |mean_neg+eps| get their cube sums
  recomputed in bf16x3 on device (column-sharded), host swaps them in.
"""

import numpy as np
from contextlib import ExitStack

V = 8192
D = 768
NCORES = 8
NK = D // 128               # 6 contraction chunks
SLOTS = 8                   # stationary 128-row slots per core
HW_ = 1024                  # moving hyper-chunk width (columns of S)
NH = V // HW_               # 8 hyper-chunks
QW = 512                    # matmul moving free size (PSUM bank)
EPS = 1e-6
LAMBDA = 0.2

REFINE_K = 128              # sensitive rows refined in bf16x3
RB = REFINE_K // 128        # stationary blocks of refined rows
SEGW = V // NCORES          # 1024-column segment per core for refinement

NT_MAX = 2 * (NH - 1)       # max tail (512-wide) ops per slot: 14

_cache = {}


def _slot_row(c: int, s: int) -> int:
    """Global 128-row block index held by (core c, slot s)."""
    return 8 * s + (c if s % 2 == 0 else 7 - c)


def _slot_pad(c: int, s: int) -> int:
    """Leading pad block-columns of slot s on core c (0..7)."""
    return _slot_row(c, s) - 8 * s


def _tf32_round(x: np.ndarray) -> np.ndarray:
    u = np.ascontiguousarray(x).view(np.uint32)
    u = (u + np.uint32(0x1000)) & np.uint32(0xFFFFE000)
    return u.view(np.float32)


def _build():
    import concourse.tile as tile
    from concourse import bacc, mybir

    F32 = mybir.dt.float32
    F32R = mybir.dt.float32r
    BF16 = mybir.dt.bfloat16
    MULT = mybir.AluOpType.mult
    ADD = mybir.AluOpType.add
    AX = mybir.AxisListType.X

    nc = bacc.Bacc("TRN2", target_bir_lowering=False, debug=False,
                   num_devices=NCORES)

    d_mov = nc.dram_tensor("mov", [D, V], F32R, kind="ExternalInput").ap()
    d_sta = nc.dram_tensor("sta", [D, SLOTS * 128], F32R,
                           kind="ExternalInput").ap()
    d_sens_hi = nc.dram_tensor("sens_hi", [D, REFINE_K], BF16,
                               kind="ExternalInput").ap()
    d_sens_lo = nc.dram_tensor("sens_lo", [D, REFINE_K], BF16,
                               kind="ExternalInput").ap()
    d_seg_hi = nc.dram_tensor("seg_hi", [D, SEGW], BF16,
                              kind="ExternalInput").ap()
    d_seg_lo = nc.dram_tensor("seg_lo", [D, SEGW], BF16,
                              kind="ExternalInput").ap()
    d_ones = nc.dram_tensor("onesv", [128, 1], F32R,
                            kind="ExternalInput").ap()

    d_rct = nc.dram_tensor("rct", [128, SLOTS, NT_MAX], F32,
                           kind="ExternalOutput").ap()
    d_rch = nc.dram_tensor("rch", [128, SLOTS, 8], F32,
                           kind="ExternalOutput").ap()
    # column-sum pieces: 8 head pieces (2x512 each) then 7 tail-chunk pieces
    d_col = nc.dram_tensor("col", [2 * (SLOTS + NH - 1), QW], F32,
                           kind="ExternalOutput").ap()
    d_rcref = nc.dram_tensor("rcref", [128, RB, SEGW // QW], F32,
                             kind="ExternalOutput").ap()

    with tile.TileContext(nc) as tc, ExitStack() as ctx:
        sta_pool = ctx.enter_context(tc.tile_pool(name="sta", bufs=1))
        ref_pool = ctx.enter_context(tc.tile_pool(name="refin", bufs=1))
        mov_pool = ctx.enter_context(tc.tile_pool(name="mov", bufs=3))
        acc_pool = ctx.enter_context(tc.tile_pool(name="acc", bufs=1))
        ps_pool = ctx.enter_context(tc.tile_pool(name="ps", bufs=4,
                                                 space="PSUM"))
        ps_col = ctx.enter_context(tc.tile_pool(name="psc", bufs=2,
                                                space="PSUM"))
        ps_ref = ctx.enter_context(tc.tile_pool(name="psr", bufs=2,
                                                space="PSUM"))
        sq_pool = ctx.enter_context(tc.tile_pool(name="sq", bufs=3))
        cb_pool = ctx.enter_context(tc.tile_pool(name="cb", bufs=3))
        out_pool = ctx.enter_context(tc.tile_pool(name="out", bufs=1))
        csb_pool = ctx.enter_context(tc.tile_pool(name="csb", bufs=2))

        # stationary slots, split per K-chunk so the first matmul's operand
        # arrives quickly
        sta = sta_pool.tile([128, NK, SLOTS * 128], F32R, tag="sta")
        sta_view = d_sta.rearrange("(c p) n -> p c n", p=128)
        for kc in range(NK):
            nc.sync.dma_start(sta[:, kc, :], sta_view[:, kc, :])

        acc_main = acc_pool.tile([128, (NH - 1) * HW_], F32R, tag="accm")
        acc_head = acc_pool.tile([128, SLOTS, HW_], F32R, tag="acch")
        rct = out_pool.tile([128, SLOTS, NT_MAX], F32, tag="rct")
        rch = out_pool.tile([128, SLOTS, 8], F32, tag="rch")
        rcref = out_pool.tile([128, RB, SEGW // QW], F32, tag="rcref")
        ones = out_pool.tile([128, 1], F32R, tag="ones")
        nc.sync.dma_start(ones[:], d_ones)

        mov_view = d_mov.rearrange("(c p) n -> p c n", p=128)
        refine_dmas_issued = False
        for hi, h in enumerate(range(NH - 1, -1, -1)):
            mov = mov_pool.tile([128, NK, HW_], F32R, tag="mov")
            nc.sync.dma_start(mov[:],
                              mov_view[:, :, h * HW_:(h + 1) * HW_])
            if not refine_dmas_issued:
                # refinement inputs are needed only at the end; queue them
                # after the first moving chunk so they don't delay the start
                refine_dmas_issued = True
                sens_hi = ref_pool.tile([128, NK, REFINE_K], BF16,
                                        tag="sens_hi")
                nc.sync.dma_start(
                    sens_hi[:], d_sens_hi.rearrange("(c p) n -> p c n", p=128))
                sens_lo = ref_pool.tile([128, NK, REFINE_K], BF16,
                                        tag="sens_lo")
                nc.sync.dma_start(
                    sens_lo[:], d_sens_lo.rearrange("(c p) n -> p c n", p=128))
                seg_hi = ref_pool.tile([128, NK, SEGW], BF16, tag="seg_hi")
                nc.sync.dma_start(
                    seg_hi[:], d_seg_hi.rearrange("(c p) n -> p c n", p=128))
                seg_lo = ref_pool.tile([128, NK, SEGW], BF16, tag="seg_lo")
                nc.sync.dma_start(
                    seg_lo[:], d_seg_lo.rearrange("(c p) n -> p c n", p=128))

            for s in range(h + 1):
                is_head = (s == h)
                for qq in range(HW_ // QW):
                    P = ps_pool.tile([128, QW], F32, tag="P")
                    for kc in range(NK):
                        nc.tensor.matmul(
                            P[:],
                            sta[:, kc, s * 128:(s + 1) * 128],
                            mov[:, kc, qq * QW:(qq + 1) * QW],
                            start=(kc == 0), stop=(kc == NK - 1))
                    sq = sq_pool.tile([128, QW], F32, tag="sq")
                    nc.scalar.square(sq[:], P[:])
                    if is_head:
                        dst = acc_head[:, s, qq * QW:(qq + 1) * QW]
                        nc.vector.scalar_tensor_tensor(
                            dst, P[:], 1.0, sq[:], MULT, MULT)
                        nc.vector.tensor_reduce(
                            rch[:, s, 4 * qq:4 * qq + 4],
                            dst.rearrange("p (b w) -> p b w", w=128),
                            axis=AX, op=ADD)
                    else:
                        # tail: row-sums into rct; S^3 into the column
                        # accumulator (slot 0 writes, others add)
                        idx = 2 * (h - s - 1) + qq
                        g0 = (h - 1) * HW_ + qq * QW
                        if s == 0:
                            nc.vector.scalar_tensor_tensor(
                                acc_main[:, g0:g0 + QW], P[:], 1.0, sq[:],
                                MULT, MULT, accum_out=rct[:, s, idx:idx + 1])
                        else:
                            cb = cb_pool.tile([128, QW], F32R, tag="cb")
                            nc.vector.scalar_tensor_tensor(
                                cb[:], P[:], 1.0, sq[:], MULT, MULT,
                                accum_out=rct[:, s, idx:idx + 1])
                            nc.gpsimd.tensor_add(
                                acc_main[:, g0:g0 + QW],
                                acc_main[:, g0:g0 + QW], cb[:])

        # bf16x3 refinement of the sensitive rows against this core's column
        # segment: hh + hl + lh accumulate in PSUM.
        pairs = [(sens_hi, seg_hi), (sens_hi, seg_lo), (sens_lo, seg_hi)]
        for b in range(RB):
            for qq in range(SEGW // QW):
                Pr = ps_ref.tile([128, QW], F32, tag="Pr")
                n_mm = len(pairs) * NK
                i_mm = 0
                for (wl, wr) in pairs:
                    for kc in range(NK):
                        nc.tensor.matmul(
                            Pr[:],
                            wl[:, kc, b * 128:(b + 1) * 128],
                            wr[:, kc, qq * QW:(qq + 1) * QW],
                            start=(i_mm == 0), stop=(i_mm == n_mm - 1))
                        i_mm += 1
                sq = sq_pool.tile([128, QW], F32, tag="sq")
                nc.scalar.square(sq[:], Pr[:])
                cb = cb_pool.tile([128, QW], F32, tag="cb")
                nc.vector.scalar_tensor_tensor(
                    cb[:], Pr[:], 1.0, sq[:], MULT, MULT,
                    accum_out=rcref[:, b, qq:qq + 1])

        # column-sum partition-reduces, deferred to the end so the in-order
        # PE queue never waits on VectorE/GpSimd mid-stream (their inputs
        # are all complete by now; refinement matmuls covered the drain).
        # Ordered so the latest-finishing accumulators (chunk 1, head 0)
        # come last.
        jobs = []
        for h in range(NH - 1, 0, -1):
            for qq in range(HW_ // QW):
                g0 = (h - 1) * HW_ + qq * QW
                jobs.append((acc_main[:, g0:g0 + QW],
                             2 * SLOTS + 2 * (h - 1) + qq))
        for h in range(NH - 1, -1, -1):
            for qq in range(HW_ // QW):
                jobs.append((acc_head[:, h, qq * QW:(qq + 1) * QW],
                             2 * h + qq))
        for src, row in jobs:
            Pc = ps_col.tile([1, QW], F32, tag="Pc")
            nc.tensor.matmul(Pc[:], ones[:], src, start=True, stop=True)
            cs = csb_pool.tile([1, QW], F32, tag="cs")
            nc.scalar.copy(cs[:], Pc[:])
            nc.sync.dma_start(d_col[row:row + 1, :], cs[:])

        nc.sync.dma_start(d_rct, rct[:])
        nc.sync.dma_start(d_rch, rch[:])
        nc.sync.dma_start(d_rcref, rcref[:])

    nc.compile()
    return nc


def _prep(t_prime: np.ndarray):
    t32 = np.ascontiguousarray(np.asarray(t_prime, dtype=np.float32))
    ss = np.einsum('ij,ij->i', t32, t32)
    norm = np.sqrt(np.maximum(ss, 1e-24))
    tn32 = t32 / norm[:, None]                       # [V, D] fp32

    # exact (fp64) mean_neg and collapse on host
    tn64 = tn32.astype(np.float64)
    s = tn64.sum(0)                                  # [D]
    rowsum = tn64 @ s                                # [V]
    diag = np.einsum('ij,ij->i', tn64, tn64)         # [V]
    mean_neg = (rowsum - diag) / (V - 1)
    den = mean_neg + EPS
    collapse = np.sum((diag - 1.0) ** 2)

    tnT = np.ascontiguousarray(tn32.T)               # [D, V]
    tnT_r = _tf32_round(tnT)                         # fp32r operand
    # predicted device diagonal (tf32 inputs, exact products)
    tr64 = tnT_r.astype(np.float64)
    diag_dev = np.einsum('ij,ij->j', tr64, tr64)     # [V]

    # sensitive rows -> bf16x3 refinement
    sens_idx = np.argsort(np.abs(den))[:REFINE_K]
    import ml_dtypes
    hi = tnT.astype(ml_dtypes.bfloat16)
    lo = (tnT - hi.astype(np.float32)).astype(ml_dtypes.bfloat16)
    hs = hi[:, sens_idx].astype(np.float64)
    ls = lo[:, sens_idx].astype(np.float64)
    diag_ref = (hs * hs + 2 * hs * ls).sum(0)        # [K]

    sens_hi = np.ascontiguousarray(hi[:, sens_idx])
    sens_lo = np.ascontiguousarray(lo[:, sens_idx])

    in_maps = []
    for c in range(NCORES):
        cols = np.concatenate([
            np.arange(128 * _slot_row(c, s), 128 * _slot_row(c, s) + 128)
            for s in range(SLOTS)])
        in_maps.append({
            "mov": tnT_r,
            "sta": np.ascontiguousarray(tnT_r[:, cols]),
            "sens_hi": sens_hi,
            "sens_lo": sens_lo,
            "seg_hi": np.ascontiguousarray(hi[:, c * SEGW:(c + 1) * SEGW]),
            "seg_lo": np.ascontiguousarray(lo[:, c * SEGW:(c + 1) * SEGW]),
            "onesv": np.ones((128, 1), np.float32),
        })
    host = dict(den=den, collapse=collapse, diag_dev=diag_dev,
                sens_idx=sens_idx, diag_ref=diag_ref)
    return in_maps, host


def _assemble(results, host):
    den = host["den"]
    rc_rows = np.zeros(V, dtype=np.float64)
    colsum = np.zeros(V, dtype=np.float64)
    for c in range(NCORES):
        rct = results[c]["rct"].astype(np.float64)   # [128, SLOTS, NT_MAX]
        rch = results[c]["rch"].astype(np.float64)   # [128, SLOTS, 8]
        col = results[c]["col"].astype(np.float64)   # [2*(SLOTS+NH-1), QW]
        for s in range(SLOTS):
            r = _slot_row(c, s)
            p = _slot_pad(c, s)
            nt = 2 * (NH - 1 - s)
            rows = slice(128 * r, 128 * r + 128)
            # tail row-sums + real head block-columns (pad..7 incl. diagonal)
            rc_rows[rows] += rct[:, s, :nt].sum(1) + rch[:, s, p:].sum(1)
            # head column-sums: strict-upper block-columns only
            # (discard pads and the diagonal block)
            hpiece = col[2 * s:2 * s + 2].reshape(8, 128)  # per block-col
            for b in range(p + 1, 8):
                colsum[128 * (8 * s + b):128 * (8 * s + b) + 128] += hpiece[b]
        # tail-chunk column-sum pieces cover global columns [1024, 8192)
        tpiece = col[2 * SLOTS:].reshape(-1)         # [7*1024]
        colsum[HW_:] += tpiece
    rc_rows += colsum
    rc_rows -= host["diag_dev"] ** 3

    # swap in refined rows: sum partial column-segments over all cores
    rc_ref = np.zeros(REFINE_K, dtype=np.float64)
    for c in range(NCORES):
        rr = results[c]["rcref"].astype(np.float64)  # [128, RB, SEGW//QW]
        rc_ref += rr.sum(axis=2).T.reshape(-1)
    rc_ref -= host["diag_ref"] ** 3
    rc_rows[host["sens_idx"]] = rc_ref

    hns = np.sum(rc_rows / den)
    return np.float32(host["collapse"] + LAMBDA * hns)


def _get_runner():
    """Build + compile the Bass module once and wrap it in a reusable
    sharded-jit callable."""
    if "runner" in _cache:
        return _cache["runner"]

    import jax
    from jax.sharding import Mesh, PartitionSpec
    from jax.experimental.shard_map import shard_map
    from concourse import bass2jax, mybir

    nc = _build()
    bass2jax.install_neuronx_cc_hook()

    partition_name = (nc.partition_id_tensor.name
                      if nc.partition_id_tensor else None)
    in_names, out_names, out_avals, zero_outs = [], [], [], []
    for alloc in nc.m.functions[0].allocations:
        if not isinstance(alloc, mybir.MemoryLocationSet):
            continue
        name = alloc.memorylocations[0].name
        if alloc.kind == "ExternalInput":
            if name != partition_name:
                in_names.append(name)
        elif alloc.kind == "ExternalOutput":
            shape = tuple(alloc.tensor_shape)
            dtype = mybir.dt.np(alloc.dtype)
            out_names.append(name)
            out_avals.append(jax.core.ShapedArray(shape, dtype))
            zero_outs.append(np.zeros(shape, dtype))
    n_params = len(in_names)
    all_names = in_names + out_names
    if partition_name is not None:
        all_names = all_names + [partition_name]

    def _body(*args):
        operands = list(args)
        if partition_name is not None:
            operands.append(bass2jax.partition_id_tensor())
        outs = bass2jax._bass_exec_p.bind(
            *operands,
            out_avals=tuple(out_avals),
            in_names=tuple(all_names),
            out_names=tuple(out_names),
            lowering_input_output_aliases=(),
            sim_require_finite=True,
            sim_require_nnan=True,
            nc=nc,
        )
        return tuple(outs)

    devices = jax.devices()[:NCORES]
    mesh = Mesh(np.asarray(devices), ("core",))
    n_outs = len(out_names)
    sharded = jax.jit(
        shard_map(_body, mesh=mesh,
                  in_specs=(PartitionSpec("core"),) * (n_params + n_outs),
                  out_specs=(PartitionSpec("core"),) * n_outs,
                  check_rep=False),
        donate_argnums=tuple(range(n_params, n_params + n_outs)),
        keep_unused=True,
    )

    def execute(in_maps, device_inputs=None):
        if device_inputs is None:
            device_inputs = [
                np.concatenate([in_maps[c][nm] for c in range(NCORES)], axis=0)
                for nm in in_names
            ]
        concat_zeros = [
            np.zeros((NCORES * z.shape[0], *z.shape[1:]), z.dtype)
            for z in zero_outs
        ]
        out_arrs = sharded(*device_inputs, *concat_zeros)
        out_arrs = [np.asarray(a) for a in out_arrs]
        return [
            {nm: out_arrs[i].reshape(NCORES, *out_avals[i].shape)[c]
             for i, nm in enumerate(out_names)}
            for c in range(NCORES)
        ]

    runner = dict(nc=nc, execute=execute, in_names=in_names,
                  out_names=out_names, sharded=sharded, zero_outs=zero_outs,
                  out_avals=out_avals, mesh=mesh)
    _cache["runner"] = runner
    return runner


def _run(t_prime: np.ndarray):
    runner = _get_runner()
    in_maps, host = _prep(np.asarray(t_prime))
    results = runner["execute"](in_maps)
    loss = _assemble(results, host)
    return loss, results


def kernel(t_prime: np.ndarray) -> np.ndarray:
    loss, _ = _run(t_prime)
    return np.asarray(loss, dtype=np.float32)


def benchmark(t_prime: np.ndarray, iters: int = 20):
    """Repeat-execute with device-resident inputs; returns per-call seconds."""
    import time
    import jax
    runner = _get_runner()
    in_maps, host = _prep(np.asarray(t_prime))
    concat = [
        np.concatenate([in_maps[c][nm] for c in range(NCORES)], axis=0)
        for nm in runner["in_names"]
    ]
    from jax.sharding import NamedSharding, PartitionSpec
    sh = NamedSharding(runner["mesh"], PartitionSpec("core"))
    dev_in = [jax.device_put(a, sh) for a in concat]
    for a in dev_in:
        a.block_until_ready()
    runner["execute"](in_maps, device_inputs=dev_in)   # warmup
    times = []
    for _ in range(iters):
        t0 = time.perf_counter()
        runner["execute"](in_maps, device_inputs=dev_in)
        times.append(time.perf_counter() - t0)
    return times


# revision 24
# speedup vs baseline: 1.0139x; 1.0139x over previous
"""TRN2 Bass kernel for nn_MFILoss_38225208934871.

loss = sum((diag(S)-1)^2) + 0.2 * sum_i [ sum_j S_off[i,j]^3 / (mean_j S_off[i,j] + 1e-6) ]
where S = t_norm @ t_norm.T, t_norm = L2-row-normalized t_prime [8192, 768].

Strategy (8 NeuronCores, SPMD, symmetric-triangle formulation):
  S is symmetric, so S^3 (elementwise) is too: row_cube[i] needs every
  unordered pair {i, j} once.  Each 128x128 block-tile (r, q) with q > r is
  computed ONCE; its row-sums go to rows of block r, and its column-sums
  (accumulated in SBUF, partition-reduced by a ones-vector matmul at the
  end) go to rows of block q.  Diagonal blocks contribute row-sums only.
  This nearly halves TensorE work vs. the full [V/8, V] slab per core.

  Uniform SPMD program: 8 stationary "slots" per core; slot s computes the
  fixed column suffix [1024*s, 8192).  The 64 row-blocks are dealt to
  (core, slot) bins serpentine-style so slot s always holds a row-block r
  in [8s, 8s+8): the <=7 leading "pad" block-columns of each slot are
  duplicated lower-triangle work whose results the host discards (the
  first 1024 columns of every slot get per-block-column row-sums and a
  separate column-accumulator strip so real/pad/diagonal parts separate).

  mean_neg (the 1e-6-offset denominator) is computed on host in fp64
  exactly; the 128 rows with smallest |mean_neg+eps| get their cube sums
  recomputed in bf16x3 on device (column-sharded), host swaps them in.
"""

import numpy as np
from contextlib import ExitStack

V = 8192
D = 768
NCORES = 8
NK = D // 128               # 6 contraction chunks
SLOTS = 8                   # stationary 128-row slots per core
HW_ = 1024                  # moving hyper-chunk width (columns of S)
NH = V // HW_               # 8 hyper-chunks
QW = 512                    # matmul moving free size (PSUM bank)
EPS = 1e-6
LAMBDA = 0.2

REFINE_K = 128              # sensitive rows refined in bf16x3
RB = REFINE_K // 128        # stationary blocks of refined rows
SEGW = V // NCORES          # 1024-column segment per core for refinement

NT_MAX = 2 * (NH - 1)       # max tail (512-wide) ops per slot: 14

_cache = {}


def _slot_row(c: int, s: int) -> int:
    """Global 128-row block index held by (core c, slot s)."""
    return 8 * s + (c if s % 2 == 0 else 7 - c)


def _slot_pad(c: int, s: int) -> int:
    """Leading pad block-columns of slot s on core c (0..7)."""
    return _slot_row(c, s) - 8 * s


def _tf32_round(x: np.ndarray) -> np.ndarray:
    u = np.ascontiguousarray(x).view(np.uint32)
    u = (u + np.uint32(0x1000)) & np.uint32(0xFFFFE000)
    return u.view(np.float32)


def _build():
    import concourse.tile as tile
    from concourse import bacc, mybir

    F32 = mybir.dt.float32
    F32R = mybir.dt.float32r
    BF16 = mybir.dt.bfloat16
    MULT = mybir.AluOpType.mult
    ADD = mybir.AluOpType.add
    AX = mybir.AxisListType.X

    nc = bacc.Bacc("TRN2", target_bir_lowering=False, debug=False,
                   num_devices=NCORES)

    d_mov = nc.dram_tensor("mov", [D, V], F32R, kind="ExternalInput").ap()
    d_sta = nc.dram_tensor("sta", [D, SLOTS * 128], F32R,
                           kind="ExternalInput").ap()
    d_sens_hi = nc.dram_tensor("sens_hi", [D, REFINE_K], BF16,
                               kind="ExternalInput").ap()
    d_sens_lo = nc.dram_tensor("sens_lo", [D, REFINE_K], BF16,
                               kind="ExternalInput").ap()
    d_seg_hi = nc.dram_tensor("seg_hi", [D, SEGW], BF16,
                              kind="ExternalInput").ap()
    d_seg_lo = nc.dram_tensor("seg_lo", [D, SEGW], BF16,
                              kind="ExternalInput").ap()
    d_ones = nc.dram_tensor("onesv", [128, 1], F32R,
                            kind="ExternalInput").ap()

    d_rct = nc.dram_tensor("rct", [128, SLOTS, NT_MAX], F32,
                           kind="ExternalOutput").ap()
    d_rch = nc.dram_tensor("rch", [128, SLOTS, 8], F32,
                           kind="ExternalOutput").ap()
    # column-sum pieces: 8 head pieces (2x512 each) then 7 tail-chunk pieces
    d_col = nc.dram_tensor("col", [2 * (SLOTS + NH - 1), QW], F32,
                           kind="ExternalOutput").ap()
    d_rcref = nc.dram_tensor("rcref", [128, RB, SEGW // QW], F32,
                             kind="ExternalOutput").ap()

    with tile.TileContext(nc) as tc, ExitStack() as ctx:
        sta_pool = ctx.enter_context(tc.tile_pool(name="sta", bufs=1))
        ref_pool = ctx.enter_context(tc.tile_pool(name="refin", bufs=1))
        mov_pool = ctx.enter_context(tc.tile_pool(name="mov", bufs=3))
        acc_pool = ctx.enter_context(tc.tile_pool(name="acc", bufs=1))
        ps_pool = ctx.enter_context(tc.tile_pool(name="ps", bufs=3,
                                                 space="PSUM"))
        ps_col = ctx.enter_context(tc.tile_pool(name="psc", bufs=3,
                                                space="PSUM"))
        ps_ref = ctx.enter_context(tc.tile_pool(name="psr", bufs=2,
                                                space="PSUM"))
        sq_pool = ctx.enter_context(tc.tile_pool(name="sq", bufs=3))
        cb_pool = ctx.enter_context(tc.tile_pool(name="cb", bufs=3))
        out_pool = ctx.enter_context(tc.tile_pool(name="out", bufs=1))
        csb_pool = ctx.enter_context(tc.tile_pool(name="csb", bufs=3))

        # stationary slots: one just-in-time DMA per slot, interleaved with
        # the moving chunks so the first matmuls start after ~1 MB of DMA
        sta = sta_pool.tile([128, NK, SLOTS * 128], F32R, tag="sta")
        sta_view = d_sta.rearrange("(c p) n -> p c n", p=128)

        acc_main = acc_pool.tile([128, (NH - 1) * HW_], F32R, tag="accm")
        acc_head = acc_pool.tile([128, SLOTS, HW_], F32R, tag="acch")
        rct = out_pool.tile([128, SLOTS, NT_MAX], F32, tag="rct")
        rch = out_pool.tile([128, SLOTS, 8], F32, tag="rch")
        rcref = out_pool.tile([128, RB, SEGW // QW], F32, tag="rcref")
        ones = out_pool.tile([128, 1], F32R, tag="ones")
        nc.sync.dma_start(ones[:], d_ones)

        mov_view = d_mov.rearrange("(c p) n -> p c n", p=128)

        def colsum_piece(src, row):
            # partition-reduce via ones-matmul, evacuate via ScalarE, then a
            # tiny DMA on the ScalarE ring (keeps the sync queue for the
            # moving stream)
            Pc = ps_col.tile([1, QW], F32, tag="Pc")
            nc.tensor.matmul(Pc[:], ones[:], src, start=True, stop=True)
            cs = csb_pool.tile([1, QW], F32, tag="cs")
            nc.scalar.copy(cs[:], Pc[:])
            nc.scalar.dma_start(d_col[row:row + 1, :], cs[:])

        def chunk_pieces(h):
            out = []
            for qq in range(HW_ // QW):
                out.append((acc_head[:, h, qq * QW:(qq + 1) * QW],
                            2 * h + qq))
            if h >= 1:
                for qq in range(HW_ // QW):
                    g0 = (h - 1) * HW_ + qq * QW
                    out.append((acc_main[:, g0:g0 + QW],
                                2 * SLOTS + 2 * (h - 1) + qq))
            return out

        # chunk order: start where per-chunk compute first covers the DMA
        # rate (chunk h engages h+1 slots), leave the light chunks for the
        # end when all data is long resident
        CHUNK_ORDER = [3, 4, 5, 6, 7, 2, 1, 0]
        sta_sent = set()
        prev_h = None
        for h in CHUNK_ORDER:
            need_sta = [s for s in range(h + 1) if s not in sta_sent]
            sta_sent.update(need_sta)
            mov = mov_pool.tile([128, NK, HW_], F32R, tag="mov")
            # interleave stationary and moving DMAs per K-chunk so the
            # first matmul only waits for ~0.8 MB instead of the full slab
            for kc in range(NK):
                if need_sta:
                    lo, hi = need_sta[0] * 128, (need_sta[-1] + 1) * 128
                    nc.sync.dma_start(sta[:, kc, lo:hi],
                                      sta_view[:, kc, lo:hi])
                nc.sync.dma_start(
                    mov[:, kc, :],
                    mov_view[:, kc, h * HW_:(h + 1) * HW_])

            for s in range(h + 1):
                is_head = (s == h)
                for qq in range(HW_ // QW):
                    P = ps_pool.tile([128, QW], F32, tag="P")
                    for kc in range(NK):
                        nc.tensor.matmul(
                            P[:],
                            sta[:, kc, s * 128:(s + 1) * 128],
                            mov[:, kc, qq * QW:(qq + 1) * QW],
                            start=(kc == 0), stop=(kc == NK - 1))
                    sq = sq_pool.tile([128, QW], F32, tag="sq")
                    nc.scalar.square(sq[:], P[:])
                    if is_head:
                        dst = acc_head[:, s, qq * QW:(qq + 1) * QW]
                        nc.vector.scalar_tensor_tensor(
                            dst, P[:], 1.0, sq[:], MULT, MULT)
                        nc.vector.tensor_reduce(
                            rch[:, s, 4 * qq:4 * qq + 4],
                            dst.rearrange("p (b w) -> p b w", w=128),
                            axis=AX, op=ADD)
                    else:
                        # tail: row-sums into rct; S^3 into the column
                        # accumulator (slot 0 writes, others add)
                        idx = 2 * (h - s - 1) + qq
                        g0 = (h - 1) * HW_ + qq * QW
                        if s == 0:
                            nc.vector.scalar_tensor_tensor(
                                acc_main[:, g0:g0 + QW], P[:], 1.0, sq[:],
                                MULT, MULT, accum_out=rct[:, s, idx:idx + 1])
                        else:
                            cb = cb_pool.tile([128, QW], F32R, tag="cb")
                            nc.vector.scalar_tensor_tensor(
                                cb[:], P[:], 1.0, sq[:], MULT, MULT,
                                accum_out=rct[:, s, idx:idx + 1])
                            nc.gpsimd.tensor_add(
                                acc_main[:, g0:g0 + QW],
                                acc_main[:, g0:g0 + QW], cb[:])
            # column-sum partition-reduces for the chunk finished one
            # iteration ago: its accumulators are long complete, so the
            # in-order PE queue never stalls on VectorE/GpSimd
            if prev_h is not None:
                for src, row in chunk_pieces(prev_h):
                    colsum_piece(src, row)
            prev_h = h

        # row-sum outputs are complete once the main loop ends
        nc.sync.dma_start(d_rct, rct[:])
        nc.sync.dma_start(d_rch, rch[:])

        # refinement inputs: queued after all moving chunks so they never
        # delay the main stream; they arrive long before they are needed
        sens_hi = ref_pool.tile([128, NK, REFINE_K], BF16, tag="sens_hi")
        nc.sync.dma_start(sens_hi[:],
                          d_sens_hi.rearrange("(c p) n -> p c n", p=128))
        sens_lo = ref_pool.tile([128, NK, REFINE_K], BF16, tag="sens_lo")
        nc.sync.dma_start(sens_lo[:],
                          d_sens_lo.rearrange("(c p) n -> p c n", p=128))
        seg_hi = ref_pool.tile([128, NK, SEGW], BF16, tag="seg_hi")
        nc.sync.dma_start(seg_hi[:],
                          d_seg_hi.rearrange("(c p) n -> p c n", p=128))
        seg_lo = ref_pool.tile([128, NK, SEGW], BF16, tag="seg_lo")
        nc.sync.dma_start(seg_lo[:],
                          d_seg_lo.rearrange("(c p) n -> p c n", p=128))

        # bf16x3 refinement of the sensitive rows against this core's column
        # segment: hh + hl + lh accumulate in PSUM.
        pairs = [(sens_hi, seg_hi), (sens_hi, seg_lo), (sens_lo, seg_hi)]
        # leftover column-sum pieces of the last chunk are interleaved
        # between refinement groups so their VectorE/GpSimd dependencies
        # complete under the refinement matmuls
        leftover = chunk_pieces(CHUNK_ORDER[-1])
        for b in range(RB):
            for qq in range(SEGW // QW):
                Pr = ps_ref.tile([128, QW], F32, tag="Pr")
                n_mm = len(pairs) * NK
                i_mm = 0
                for (wl, wr) in pairs:
                    for kc in range(NK):
                        nc.tensor.matmul(
                            Pr[:],
                            wl[:, kc, b * 128:(b + 1) * 128],
                            wr[:, kc, qq * QW:(qq + 1) * QW],
                            start=(i_mm == 0), stop=(i_mm == n_mm - 1))
                        i_mm += 1
                sq = sq_pool.tile([128, QW], F32, tag="sq")
                nc.scalar.square(sq[:], Pr[:])
                cb = cb_pool.tile([128, QW], F32, tag="cb")
                nc.vector.scalar_tensor_tensor(
                    cb[:], Pr[:], 1.0, sq[:], MULT, MULT,
                    accum_out=rcref[:, b, qq:qq + 1])
                if qq > 0 or b > 0:
                    if leftover:
                        colsum_piece(*leftover.pop(0))
        for src, row in leftover:
            colsum_piece(src, row)

        nc.sync.dma_start(d_rcref, rcref[:])

    nc.compile()
    return nc


def _prep(t_prime: np.ndarray):
    t32 = np.ascontiguousarray(np.asarray(t_prime, dtype=np.float32))
    ss = np.einsum('ij,ij->i', t32, t32)
    norm = np.sqrt(np.maximum(ss, 1e-24))
    tn32 = t32 / norm[:, None]                       # [V, D] fp32

    # exact (fp64) mean_neg and collapse on host
    tn64 = tn32.astype(np.float64)
    s = tn64.sum(0)                                  # [D]
    rowsum = tn64 @ s                                # [V]
    diag = np.einsum('ij,ij->i', tn64, tn64)         # [V]
    mean_neg = (rowsum - diag) / (V - 1)
    den = mean_neg + EPS
    collapse = np.sum((diag - 1.0) ** 2)

    tnT = np.ascontiguousarray(tn32.T)               # [D, V]
    tnT_r = _tf32_round(tnT)                         # fp32r operand
    # predicted device diagonal (tf32 inputs, exact products)
    tr64 = tnT_r.astype(np.float64)
    diag_dev = np.einsum('ij,ij->j', tr64, tr64)     # [V]

    # sensitive rows -> bf16x3 refinement
    sens_idx = np.argsort(np.abs(den))[:REFINE_K]
    import ml_dtypes
    hi = tnT.astype(ml_dtypes.bfloat16)
    lo = (tnT - hi.astype(np.float32)).astype(ml_dtypes.bfloat16)
    hs = hi[:, sens_idx].astype(np.float64)
    ls = lo[:, sens_idx].astype(np.float64)
    diag_ref = (hs * hs + 2 * hs * ls).sum(0)        # [K]

    sens_hi = np.ascontiguousarray(hi[:, sens_idx])
    sens_lo = np.ascontiguousarray(lo[:, sens_idx])

    in_maps = []
    for c in range(NCORES):
        cols = np.concatenate([
            np.arange(128 * _slot_row(c, s), 128 * _slot_row(c, s) + 128)
            for s in range(SLOTS)])
        in_maps.append({
            "mov": tnT_r,
            "sta": np.ascontiguousarray(tnT_r[:, cols]),
            "sens_hi": sens_hi,
            "sens_lo": sens_lo,
            "seg_hi": np.ascontiguousarray(hi[:, c * SEGW:(c + 1) * SEGW]),
            "seg_lo": np.ascontiguousarray(lo[:, c * SEGW:(c + 1) * SEGW]),
            "onesv": np.ones((128, 1), np.float32),
        })
    host = dict(den=den, collapse=collapse, diag_dev=diag_dev,
                sens_idx=sens_idx, diag_ref=diag_ref)
    return in_maps, host


def _assemble(results, host):
    den = host["den"]
    rc_rows = np.zeros(V, dtype=np.float64)
    colsum = np.zeros(V, dtype=np.float64)
    for c in range(NCORES):
        rct = results[c]["rct"].astype(np.float64)   # [128, SLOTS, NT_MAX]
        rch = results[c]["rch"].astype(np.float64)   # [128, SLOTS, 8]
        col = results[c]["col"].astype(np.float64)   # [2*(SLOTS+NH-1), QW]
        for s in range(SLOTS):
            r = _slot_row(c, s)
            p = _slot_pad(c, s)
            nt = 2 * (NH - 1 - s)
            rows = slice(128 * r, 128 * r + 128)
            # tail row-sums + real head block-columns (pad..7 incl. diagonal)
            rc_rows[rows] += rct[:, s, :nt].sum(1) + rch[:, s, p:].sum(1)
            # head column-sums: strict-upper block-columns only
            # (discard pads and the diagonal block)
            hpiece = col[2 * s:2 * s + 2].reshape(8, 128)  # per block-col
            for b in range(p + 1, 8):
                colsum[128 * (8 * s + b):128 * (8 * s + b) + 128] += hpiece[b]
        # tail-chunk column-sum pieces cover global columns [1024, 8192)
        tpiece = col[2 * SLOTS:].reshape(-1)         # [7*1024]
        colsum[HW_:] += tpiece
    rc_rows += colsum
    rc_rows -= host["diag_dev"] ** 3

    # swap in refined rows: sum partial column-segments over all cores
    rc_ref = np.zeros(REFINE_K, dtype=np.float64)
    for c in range(NCORES):
        rr = results[c]["rcref"].astype(np.float64)  # [128, RB, SEGW//QW]
        rc_ref += rr.sum(axis=2).T.reshape(-1)
    rc_ref -= host["diag_ref"] ** 3
    rc_rows[host["sens_idx"]] = rc_ref

    hns = np.sum(rc_rows / den)
    return np.float32(host["collapse"] + LAMBDA * hns)


def _get_runner():
    """Build + compile the Bass module once and wrap it in a reusable
    sharded-jit callable."""
    if "runner" in _cache:
        return _cache["runner"]

    import jax
    from jax.sharding import Mesh, PartitionSpec
    from jax.experimental.shard_map import shard_map
    from concourse import bass2jax, mybir

    nc = _build()
    bass2jax.install_neuronx_cc_hook()

    partition_name = (nc.partition_id_tensor.name
                      if nc.partition_id_tensor else None)
    in_names, out_names, out_avals, zero_outs = [], [], [], []
    for alloc in nc.m.functions[0].allocations:
        if not isinstance(alloc, mybir.MemoryLocationSet):
            continue
        name = alloc.memorylocations[0].name
        if alloc.kind == "ExternalInput":
            if name != partition_name:
                in_names.append(name)
        elif alloc.kind == "ExternalOutput":
            shape = tuple(alloc.tensor_shape)
            dtype = mybir.dt.np(alloc.dtype)
            out_names.append(name)
            out_avals.append(jax.core.ShapedArray(shape, dtype))
            zero_outs.append(np.zeros(shape, dtype))
    n_params = len(in_names)
    all_names = in_names + out_names
    if partition_name is not None:
        all_names = all_names + [partition_name]

    def _body(*args):
        operands = list(args)
        if partition_name is not None:
            operands.append(bass2jax.partition_id_tensor())
        outs = bass2jax._bass_exec_p.bind(
            *operands,
            out_avals=tuple(out_avals),
            in_names=tuple(all_names),
            out_names=tuple(out_names),
            lowering_input_output_aliases=(),
            sim_require_finite=True,
            sim_require_nnan=True,
            nc=nc,
        )
        return tuple(outs)

    devices = jax.devices()[:NCORES]
    mesh = Mesh(np.asarray(devices), ("core",))
    n_outs = len(out_names)
    sharded = jax.jit(
        shard_map(_body, mesh=mesh,
                  in_specs=(PartitionSpec("core"),) * (n_params + n_outs),
                  out_specs=(PartitionSpec("core"),) * n_outs,
                  check_rep=False),
        donate_argnums=tuple(range(n_params, n_params + n_outs)),
        keep_unused=True,
    )

    def execute(in_maps, device_inputs=None):
        if device_inputs is None:
            device_inputs = [
                np.concatenate([in_maps[c][nm] for c in range(NCORES)], axis=0)
                for nm in in_names
            ]
        concat_zeros = [
            np.zeros((NCORES * z.shape[0], *z.shape[1:]), z.dtype)
            for z in zero_outs
        ]
        out_arrs = sharded(*device_inputs, *concat_zeros)
        out_arrs = [np.asarray(a) for a in out_arrs]
        return [
            {nm: out_arrs[i].reshape(NCORES, *out_avals[i].shape)[c]
             for i, nm in enumerate(out_names)}
            for c in range(NCORES)
        ]

    runner = dict(nc=nc, execute=execute, in_names=in_names,
                  out_names=out_names, sharded=sharded, zero_outs=zero_outs,
                  out_avals=out_avals, mesh=mesh)
    _cache["runner"] = runner
    return runner


def _run(t_prime: np.ndarray):
    runner = _get_runner()
    in_maps, host = _prep(np.asarray(t_prime))
    results = runner["execute"](in_maps)
    loss = _assemble(results, host)
    return loss, results


def kernel(t_prime: np.ndarray) -> np.ndarray:
    loss, _ = _run(t_prime)
    return np.asarray(loss, dtype=np.float32)


def benchmark(t_prime: np.ndarray, iters: int = 20):
    """Repeat-execute with device-resident inputs; returns per-call seconds."""
    import time
    import jax
    runner = _get_runner()
    in_maps, host = _prep(np.asarray(t_prime))
    concat = [
        np.concatenate([in_maps[c][nm] for c in range(NCORES)], axis=0)
        for nm in runner["in_names"]
    ]
    from jax.sharding import NamedSharding, PartitionSpec
    sh = NamedSharding(runner["mesh"], PartitionSpec("core"))
    dev_in = [jax.device_put(a, sh) for a in concat]
    for a in dev_in:
        a.block_until_ready()
    runner["execute"](in_maps, device_inputs=dev_in)   # warmup
    times = []
    for _ in range(iters):
        t0 = time.perf_counter()
        runner["execute"](in_maps, device_inputs=dev_in)
        times.append(time.perf_counter() - t0)
    return times
